# revision 10
# baseline (speedup 1.0000x reference)
"""Single-head causal attention (B=8, T=2048, C=1024, head_dim=64) on 8 TRN2 NeuronCores.

Sharding: data-parallel over batch -- one batch element per core, qkv weights
replicated. Host prep per core: x[b] is transposed to [C, T] and cast to fp16
(PE streams fp16 at 1 cycle/row vs 4 for fp32; fp16's 11-bit mantissa keeps the
end-to-end error ~1e-3, and all PSUM accumulation stays fp32). W is pre-packed
on host into the SBUF chunk layout so its DMA is a single contiguous transfer.

Device layout (everything kept transposed so no P-tile transposes are needed):
  kqT  = Wkq^T x^T + b_kq     [128, T]   (k rows 0:64, q rows 64:128)
  qT   = rows 64:128 of kqT moved to base partition 0 via a PE row-extract
  vT   = Wv^T x^T + b_v       [64, T] -> v1 [s, 65] via DMA-transpose (+ones col)
  ST_j = K_j Q^T              [128 s, t] per 128-row s-chunk, causal t >= s only
  P^T  = exp(0.125 * ST)      masked upper-tri on the diagonal block
  outT[g] += v1_j^T P^T_j     [65, 512] per 512-col t-group (row 64 = denom)
  epilogue (interleaved, per finished t-group): DMA-transpose outT -> [t, 65],
  out = outT[:, :64] * recip(denom)  (biases folded into the projection via an
  augmented ones-row K=1 contraction chunk)
"""

import numpy as np

import concourse.bass as bass
import concourse.mybir as mybir
from concourse import bacc
from concourse.bass import ts
from concourse.bass_utils import run_bass_kernel_spmd
from concourse.masks import make_upper_triangular
from concourse.tile import TileContext

B, T, C = 8, 2048, 1024
HD = 64
N_CORES = 8
NJ = C // 128  # contraction chunks for the qkv projection
NT = T // 128  # 128-row tiles along T
NG = T // 512  # 512-col groups along T
FP16 = mybir.dt.float16
F32 = mybir.dt.float32
EXP = mybir.ActivationFunctionType.Exp


def build_nc() -> bass.Bass:
    nc = bacc.Bacc(None, target_bir_lowering=False)
    # w is pre-packed on host: [128, NJ*192] with w[p, j*192+m] = W[j*128+p, m]
    xt = nc.declare_dram_parameter("xt", [C, T], FP16, isOutput=False)
    w = nc.declare_dram_parameter("w", [128, NJ * 3 * HD], FP16, isOutput=False)
    wb = nc.declare_dram_parameter("wb", [1, 3 * HD], FP16, isOutput=False)
    out = nc.declare_dram_parameter("out", [T, HD], F32, isOutput=True)

    with TileContext(nc) as tc:
        with (
            tc.tile_pool(name="consts", bufs=1) as consts,
            tc.tile_pool(name="xtp", bufs=NJ) as xtp,
            tc.tile_pool(name="kqv", bufs=1) as kqv,
            tc.tile_pool(name="ptp", bufs=3) as ptp,
            tc.tile_pool(name="epi", bufs=3) as epi,
            tc.tile_pool(name="pso", bufs=4, space=bass.MemorySpace.PSUM) as pso,
            tc.tile_pool(name="pst", bufs=2, space=bass.MemorySpace.PSUM) as pst,
        ):
            # --- constants (w first: every matmul needs it) ---
            w_sb = consts.tile([128, NJ, 3 * HD], FP16)
            nc.sync.dma_start(out=w_sb[:], in_=w[:, :].rearrange("p (n m) -> p n m", n=NJ))
            wb_sb = consts.tile([1, 3 * HD], FP16)
            nc.sync.dma_start(out=wb_sb[:], in_=wb[:, :])
            ones_sb = consts.tile([1, T], FP16)
            nc.vector.memset(ones_sb[:], 1.0)
            mask_sb = consts.tile([128, 128], FP16)
            make_upper_triangular(nc, mask_sb[:], val=1.0, diag=True)
            # sel[c, m] = 1 iff c == m + 64: extracts partitions 64:128 -> 0:64
            sel = consts.tile([128, 64], FP16)
            nc.gpsimd.memset(sel[:], 0.0)
            nc.gpsimd.affine_select(
                out=sel[:], in_=sel[:],
                compare_op=mybir.AluOpType.not_equal,
                fill=1.0, base=-64, pattern=[[-1, 64]], channel_multiplier=1,
            )

            # --- load x^T in 128-partition chunks ---
            xts = []
            for j in range(NJ):
                xt_t = xtp.tile([128, T], FP16, tag="xt")
                nc.sync.dma_start(out=xt_t[:], in_=xt[ts(j, 128), :])
                xts.append(xt_t)

            # --- qkv projection in two passes of two 512-col groups each,
            # so only 4 PSUM banks are held at a time ---
            kqT = kqv.tile([128, T], FP16)
            vT = kqv.tile([64, T], FP16)
            qT = kqv.tile([64, T], FP16)
            # v in [s, hd] layout plus a ones column (softmax-denominator trick);
            # row stride 80 elems keeps each slice 32B-aligned for DMA-transpose
            v1 = kqv.tile([128, NT, 80], FP16)
            for half in range(2):
                for n in (2 * half, 2 * half + 1):
                    kq_acc = pso.tile([128, 512], F32, tag="o", name=f"kq_acc{n}")
                    v_acc = pso.tile([64, 512], F32, tag="o", name=f"v_acc{n}")
                    for j in range(NJ):
                        nc.tensor.matmul(
                            kq_acc[:], w_sb[:, j, 0:128], xts[j][:, ts(n, 512)],
                            start=(j == 0), stop=False,
                        )
                        nc.tensor.matmul(
                            v_acc[:], w_sb[:, j, 128:192], xts[j][:, ts(n, 512)],
                            start=(j == 0), stop=False,
                        )
                    # bias via an augmented K=1 chunk: ones row x bias row
                    nc.tensor.matmul(
                        kq_acc[:], wb_sb[:, 0:128], ones_sb[:, ts(n, 512)],
                        start=False, stop=True,
                    )
                    nc.tensor.matmul(
                        v_acc[:], wb_sb[:, 128:192], ones_sb[:, ts(n, 512)],
                        start=False, stop=True,
                    )
                    nc.vector.tensor_copy(kqT[:, ts(n, 512)], kq_acc[:])
                    nc.vector.tensor_copy(vT[:, ts(n, 512)], v_acc[:])
                    # q rows of kqT must move to base partition 0: PE row-extract
                    qp = pso.tile([64, 512], F32, tag="o", name=f"qp{n}")
                    nc.tensor.matmul(qp[:], sel[:], kqT[:, ts(n, 512)], start=True, stop=True)
                    nc.vector.tensor_copy(qT[:, ts(n, 512)], qp[:])
                    # v1 tiles for this 512-col group via DMA-transpose
                    for i in range(4 * n, 4 * n + 4):
                        nc.sync.dma_start(
                            out=v1[:, i, 0:HD], in_=vT[0:64, ts(i, 128)], transpose=True,
                        )
                        nc.vector.memset(v1[:, i, HD:HD + 1], 1.0)

            # --- attention; epilogue per t-group interleaved into the loop ---
            outT_acc = [pso.tile([65, 512], F32, tag="o", name=f"outT_acc{g}") for g in range(NG)]
            for j in range(NT):
                t0 = 128 * j
                ptj = ptp.tile([128, T], FP16, tag="pt")
                # scores^T in up-to-1024-col PSUM chunks, one exp per chunk
                for h in range(t0 // 1024, 2):
                    base = 1024 * h
                    a, b2 = max(t0, base), base + 1024
                    stp = pst.tile([128, 1024], F32, tag="st")
                    for m in range(a // 512, b2 // 512):
                        pa, pb = max(a, 512 * m), 512 * (m + 1)
                        nc.tensor.matmul(
                            stp[:, pa - base:pb - base], kqT[0:64, ts(j, 128)], qT[:, pa:pb],
                            start=True, stop=True,
                        )
                    nc.scalar.activation(ptj[:, a:b2], stp[:, a - base:], EXP, scale=0.125)
                # zero the below-diagonal (s > t) entries of the diagonal block
                nc.vector.tensor_mul(ptj[:, ts(j, 128)], ptj[:, ts(j, 128)], mask_sb[:])
                for g in range(j // 4, NG):
                    a, b2 = max(t0, 512 * g), 512 * (g + 1)
                    nc.tensor.matmul(
                        outT_acc[g][:, a - 512 * g:512], v1[:, j, 0:65], ptj[:, a:b2],
                        start=(j == 0), stop=(j == 4 * g + 3),
                    )
                if j % 4 == 3:
                    # epilogue for finished t-group g: DMA-transpose outT to
                    # [t, 65], then divide by the softmax denominator (col 64)
                    g = j // 4
                    eo = epi.tile([128, 512], FP16, tag="eo")
                    nc.vector.memset(eo[64:128, :], 0.0)
                    nc.vector.tensor_copy(eo[0:65, :], outT_acc[g][:])
                    for l in range(4):
                        i = 4 * g + l
                        tpb = epi.tile([128, 128], FP16, tag="tpb")
                        nc.sync.dma_start(out=tpb[:], in_=eo[:, ts(l, 128)], transpose=True)
                        rcp = epi.tile([128, 1], F32, tag="rcp")
                        nc.vector.reciprocal(rcp[:], tpb[:, HD:HD + 1])
                        ob = epi.tile([128, HD], F32, tag="ob")
                        nc.vector.tensor_scalar_mul(ob[:], tpb[:, 0:HD], rcp[:])
                        nc.sync.dma_start(out=out[ts(i, 128), :], in_=ob[:])
    nc.compile()
    return nc


_NC_CACHE = None


def _get_nc() -> bass.Bass:
    global _NC_CACHE
    if _NC_CACHE is None:
        _NC_CACHE = build_nc()
    return _NC_CACHE


def make_in_maps(x: np.ndarray, W: np.ndarray, b: np.ndarray) -> list[dict]:
    # pack W into the on-chip chunk layout: [128, NJ*192], contiguous DMA
    w16 = np.ascontiguousarray(
        W.astype(np.float16).reshape(NJ, 128, 3 * HD).transpose(1, 0, 2).reshape(128, NJ * 3 * HD)
    )
    wb16 = np.ascontiguousarray(b.astype(np.float16).reshape(1, 3 * HD))
    in_maps = []
    for core in range(N_CORES):
        xt = np.ascontiguousarray(x[core].astype(np.float16).T)
        in_maps.append({"xt": xt, "w": w16, "wb": wb16})
    return in_maps


def run(x, W, b, trace: bool = False):
    """Returns (output [B, T, HD] fp32, BassKernelResults)."""
    x, W, b = np.asarray(x), np.asarray(W), np.asarray(b)
    nc = _get_nc()
    res = run_bass_kernel_spmd(nc, make_in_maps(x, W, b), list(range(N_CORES)), trace=trace)
    out = np.stack([res.results[i]["out"] for i in range(N_CORES)], axis=0)
    return out.astype(np.float32), res


def kernel(x, W, b) -> np.ndarray:
    out, _ = run(x, W, b)
    return out


# revision 13
# speedup vs baseline: 1.2432x; 1.2432x over previous
"""Single-head causal attention (B=8, T=2048, C=1024, head_dim=64) on 8 TRN2 NeuronCores.

Sharding: data-parallel over batch -- one batch element per core, qkv weights
replicated. Host prep per core: x[b] is transposed to [C, T] and cast to fp16
(PE streams fp16 at 1 cycle/row vs 4 for fp32; fp16's 11-bit mantissa keeps the
end-to-end error ~1e-3, and all PSUM accumulation stays fp32). W is pre-packed
on host into the SBUF chunk layout so its DMA is one contiguous transfer; the
tiny constant matrices (causal mask, row-extract selector, identity) also come
from host so no GPSIMD library load lands in the critical preamble.

Device schedule:
  kqT  = Wkq^T x^T + b_kq     [128, T]  (k rows 0:64, q rows 64:128; biases via
                                         an augmented ones-row K=1 chunk)
  qT   = rows 64:128 of kqT moved to base partition 0 via a PE row-extract
  vT   = Wv^T x^T + b_v       [128, T] (rows 64:128 zero) -> v1 [s, 65] tiles
                                        via PE transpose (+ones denom column)
  per s-chunk j:  ST_j = K_j Q^T  [128 s, t], t >= 128j only (causal),
                  P^T = exp(0.125*ST) in up-to-1024-col chunks, diagonal block
                  masked upper-tri, then  out_acc[i] += P^T_j[:, i]^T [v|1]_j
                  for every 128-row t-tile i >= j (output lands in NORMAL
                  [t, 65] orientation; 4 accumulators packed per PSUM bank)
  after j = 4m+3: t-tiles 4m..4m+3 are complete -> divide by the denominator
                  column and DMA out. No output transposes needed anywhere.
"""

import numpy as np

import concourse.bass as bass
import concourse.mybir as mybir
from concourse import bacc
from concourse.bass import ts
from concourse.bass_utils import run_bass_kernel_spmd
from concourse.tile import TileContext

B, T, C = 8, 2048, 1024
HD = 64
N_CORES = 8
NJ = C // 128  # contraction chunks for the qkv projection
NT = T // 128  # 128-row tiles along T
NG = T // 512  # 512-col groups along T
FP16 = mybir.dt.float16
F32 = mybir.dt.float32
EXP = mybir.ActivationFunctionType.Exp


def build_nc() -> bass.Bass:
    nc = bacc.Bacc(None, target_bir_lowering=False)
    # w is pre-packed on host: [128, NJ*192] with w[p, j*192+m] = W[j*128+p, m]
    xt = nc.declare_dram_parameter("xt", [C, T], FP16, isOutput=False)
    w = nc.declare_dram_parameter("w", [128, NJ * 3 * HD], FP16, isOutput=False)
    wb = nc.declare_dram_parameter("wb", [1, 3 * HD], FP16, isOutput=False)
    msk = nc.declare_dram_parameter("msk", [128, 128], FP16, isOutput=False)
    sel = nc.declare_dram_parameter("sel", [128, 64], FP16, isOutput=False)
    idh = nc.declare_dram_parameter("idh", [128, 128], FP16, isOutput=False)
    out = nc.declare_dram_parameter("out", [T, HD], F32, isOutput=True)

    with TileContext(nc) as tc:
        with (
            tc.tile_pool(name="consts", bufs=1) as consts,
            tc.tile_pool(name="xtp", bufs=NJ) as xtp,
            tc.tile_pool(name="kqv", bufs=1) as kqv,
            tc.tile_pool(name="ptp", bufs=3) as ptp,
            tc.tile_pool(name="epi", bufs=4) as epi,
            tc.tile_pool(name="pso", bufs=4, space=bass.MemorySpace.PSUM) as pso,
            tc.tile_pool(name="pst", bufs=2, space=bass.MemorySpace.PSUM) as pst,
        ):
            # --- constants (w first: every matmul needs it) ---
            w_sb = consts.tile([128, NJ, 3 * HD], FP16)
            nc.sync.dma_start(out=w_sb[:], in_=w[:, :].rearrange("p (n m) -> p n m", n=NJ))
            wb_sb = consts.tile([1, 3 * HD], FP16)
            nc.sync.dma_start(out=wb_sb[:], in_=wb[:, :])
            msk_sb = consts.tile([128, 128], FP16)
            nc.sync.dma_start(out=msk_sb[:], in_=msk[:, :])
            sel_sb = consts.tile([128, 64], FP16)
            nc.sync.dma_start(out=sel_sb[:], in_=sel[:, :])
            idh_sb = consts.tile([128, 128], FP16)
            nc.sync.dma_start(out=idh_sb[:], in_=idh[:, :])
            ones_sb = consts.tile([1, T], FP16)
            nc.vector.memset(ones_sb[:], 1.0)

            # --- load x^T in 128-partition chunks ---
            xts = []
            for j in range(NJ):
                xt_t = xtp.tile([128, T], FP16, tag="xt")
                nc.sync.dma_start(out=xt_t[:], in_=xt[ts(j, 128), :])
                xts.append(xt_t)

            # --- qkv projection, two 512-col groups in flight (4 PSUM banks) ---
            kqT = kqv.tile([128, T], FP16)
            vT = kqv.tile([128, T], FP16)  # rows 64:128 zero-padded for transpose
            qT = kqv.tile([64, T], FP16)
            v1 = kqv.tile([128, NT, 80], FP16)  # [s, hd | ones | pad] per t-tile
            nc.vector.memset(vT[64:128, :], 0.0)
            for n in range(NG):
                kq_acc = pso.tile([128, 512], F32, tag="o", name=f"kq_acc{n}")
                v_acc = pso.tile([64, 512], F32, tag="o", name=f"v_acc{n}")
                for j in range(NJ):
                    nc.tensor.matmul(
                        kq_acc[:], w_sb[:, j, 0:128], xts[j][:, ts(n, 512)],
                        start=(j == 0), stop=False,
                    )
                    nc.tensor.matmul(
                        v_acc[:], w_sb[:, j, 128:192], xts[j][:, ts(n, 512)],
                        start=(j == 0), stop=False,
                    )
                # bias via an augmented K=1 chunk: ones row x bias row
                nc.tensor.matmul(
                    kq_acc[:], wb_sb[:, 0:128], ones_sb[:, ts(n, 512)],
                    start=False, stop=True,
                )
                nc.tensor.matmul(
                    v_acc[:], wb_sb[:, 128:192], ones_sb[:, ts(n, 512)],
                    start=False, stop=True,
                )
                nc.vector.tensor_copy(kqT[:, ts(n, 512)], kq_acc[:])
                nc.vector.tensor_copy(vT[0:64, ts(n, 512)], v_acc[:])
                # q rows of kqT must move to base partition 0: PE row-extract
                qp = pso.tile([64, 512], F32, tag="o", name=f"qp{n}")
                nc.tensor.matmul(qp[:], sel_sb[:], kqT[:, ts(n, 512)], start=True, stop=True)
                nc.vector.tensor_copy(qT[:, ts(n, 512)], qp[:])
                # v1 tiles for this 512-col group via PE transpose of padded vT
                for i in range(4 * n, 4 * n + 4):
                    tpv = pst.tile([128, 128], FP16, tag="st", name=f"tpv{i}")
                    nc.tensor.transpose(tpv[:], vT[:, ts(i, 128)], idh_sb[:])
                    nc.vector.tensor_copy(v1[:, i, 0:HD], tpv[:, 0:HD])
                    nc.vector.memset(v1[:, i, HD:HD + 1], 1.0)

            # --- attention: out_acc[i] accumulates in normal [t, 65] layout,
            # 4 accumulators packed per PSUM bank ---
            out_acc = [pso.tile([128, 4, 65], F32, tag="o", name=f"out_acc{m}") for m in range(4)]
            for j in range(NT):
                t0 = 128 * j
                ptj = ptp.tile([128, T], FP16, tag="pt")
                # scores^T in up-to-1024-col PSUM chunks, one exp per chunk
                for h in range(t0 // 1024, 2):
                    base = 1024 * h
                    a, b2 = max(t0, base), base + 1024
                    stp = pst.tile([128, 1024], F32, tag="st")
                    for m in range(a // 512, b2 // 512):
                        pa, pb = max(a, 512 * m), 512 * (m + 1)
                        nc.tensor.matmul(
                            stp[:, pa - base:pb - base], kqT[0:64, ts(j, 128)], qT[:, pa:pb],
                            start=True, stop=True,
                        )
                    nc.scalar.activation(ptj[:, a:b2], stp[:, a - base:], EXP, scale=0.125)
                # zero the below-diagonal (s > t) entries of the diagonal block
                nc.vector.tensor_mul(ptj[:, ts(j, 128)], ptj[:, ts(j, 128)], msk_sb[:])
                # direct PV: P^T block (i, j) is the stationary operand
                for i in range(j, NT):
                    nc.tensor.matmul(
                        out_acc[i // 4][:, i % 4, :], ptj[:, ts(i, 128)], v1[:, j, 0:65],
                        start=(j == 0 and i % 4 == 0), stop=(i == j and i % 4 == 3),
                        skip_group_check=True,
                    )
                if j % 4 == 3:
                    # t-tiles 4m..4m+3 are complete: normalize and store
                    m = j // 4
                    for l in range(4):
                        i = 4 * m + l
                        rcp = epi.tile([128, 1], F32, tag="rcp")
                        nc.vector.reciprocal(rcp[:], out_acc[m][:, l, HD:HD + 1])
                        ob = epi.tile([128, HD], F32, tag="ob")
                        nc.vector.tensor_scalar_mul(ob[:], out_acc[m][:, l, 0:HD], rcp[:])
                        nc.sync.dma_start(out=out[ts(i, 128), :], in_=ob[:])
    nc.compile()
    return nc


_NC_CACHE = None


def _get_nc() -> bass.Bass:
    global _NC_CACHE
    if _NC_CACHE is None:
        _NC_CACHE = build_nc()
    return _NC_CACHE


def make_in_maps(x: np.ndarray, W: np.ndarray, b: np.ndarray) -> list[dict]:
    # pack W into the on-chip chunk layout: [128, NJ*192], contiguous DMA
    w16 = np.ascontiguousarray(
        W.astype(np.float16).reshape(NJ, 128, 3 * HD).transpose(1, 0, 2).reshape(128, NJ * 3 * HD)
    )
    wb16 = np.ascontiguousarray(b.astype(np.float16).reshape(1, 3 * HD))
    msk = np.triu(np.ones((128, 128), dtype=np.float16))        # keep s <= t
    sel = np.zeros((128, 64), dtype=np.float16)                  # rows 64:128 -> 0:64
    sel[np.arange(64) + 64, np.arange(64)] = 1.0
    idh = np.eye(128, dtype=np.float16)
    in_maps = []
    for core in range(N_CORES):
        xtc = np.ascontiguousarray(x[core].astype(np.float16).T)
        in_maps.append({"xt": xtc, "w": w16, "wb": wb16, "msk": msk, "sel": sel, "idh": idh})
    return in_maps


def run(x, W, b, trace: bool = False):
    """Returns (output [B, T, HD] fp32, BassKernelResults)."""
    x, W, b = np.asarray(x), np.asarray(W), np.asarray(b)
    nc = _get_nc()
    res = run_bass_kernel_spmd(nc, make_in_maps(x, W, b), list(range(N_CORES)), trace=trace)
    out = np.stack([res.results[i]["out"] for i in range(N_CORES)], axis=0)
    return out.astype(np.float32), res


def kernel(x, W, b) -> np.ndarray:
    out, _ = run(x, W, b)
    return out


# revision 14
# speedup vs baseline: 1.2502x; 1.0056x over previous
"""Single-head causal attention (B=8, T=2048, C=1024, head_dim=64) on 8 TRN2 NeuronCores.

Sharding: data-parallel over batch -- one batch element per core, qkv weights
replicated. Host prep per core: x[b] is transposed to [C, T] and cast to fp16
(PE streams fp16 at 1 cycle/row vs 4 for fp32; fp16's 11-bit mantissa keeps the
end-to-end error ~1e-3, and all PSUM accumulation stays fp32). W is pre-packed
on host into the SBUF chunk layout so its DMA is one contiguous transfer; the
tiny constant matrices (causal mask, row-extract selector, identity) also come
from host so no GPSIMD library load lands in the critical preamble.

Device schedule:
  kqT  = Wkq^T x^T + b_kq     [128, T]  (k rows 0:64, q rows 64:128; biases via
                                         an augmented ones-row K=1 chunk)
  qT   = rows 64:128 of kqT moved to base partition 0 via a PE row-extract
  vT   = Wv^T x^T + b_v       [128, T] (rows 64:128 zero) -> v1 [s, 65] tiles
                                        via PE transpose (+ones denom column)
  per s-chunk j:  ST_j = K_j Q^T  [128 s, t], t >= 128j only (causal),
                  P^T = exp(0.125*ST) in up-to-1024-col chunks, diagonal block
                  masked upper-tri, then  out_acc[i] += P^T_j[:, i]^T [v|1]_j
                  for every 128-row t-tile i >= j (output lands in NORMAL
                  [t, 65] orientation; 4 accumulators packed per PSUM bank)
  after j = 4m+3: t-tiles 4m..4m+3 are complete -> divide by the denominator
                  column and DMA out. No output transposes needed anywhere.
"""

import numpy as np

import concourse.bass as bass
import concourse.mybir as mybir
from concourse import bacc
from concourse.bass import ts
from concourse.bass_utils import run_bass_kernel_spmd
from concourse.tile import TileContext

B, T, C = 8, 2048, 1024
HD = 64
N_CORES = 8
NJ = C // 128  # contraction chunks for the qkv projection
NT = T // 128  # 128-row tiles along T
NG = T // 512  # 512-col groups along T
FP16 = mybir.dt.float16
F32 = mybir.dt.float32
EXP = mybir.ActivationFunctionType.Exp


def build_nc() -> bass.Bass:
    nc = bacc.Bacc(None, target_bir_lowering=False)
    # w is pre-packed on host: [128, NJ*192] with w[p, j*192+m] = W[j*128+p, m]
    xt = nc.declare_dram_parameter("xt", [C, T], FP16, isOutput=False)
    w = nc.declare_dram_parameter("w", [128, NJ * 3 * HD], FP16, isOutput=False)
    bkq = nc.declare_dram_parameter("bkq", [128, 1], F32, isOutput=False)
    bv = nc.declare_dram_parameter("bv", [64, 1], F32, isOutput=False)
    msk = nc.declare_dram_parameter("msk", [128, 128], FP16, isOutput=False)
    sel = nc.declare_dram_parameter("sel", [128, 64], FP16, isOutput=False)
    idh = nc.declare_dram_parameter("idh", [128, 128], FP16, isOutput=False)
    out = nc.declare_dram_parameter("out", [T, HD], F32, isOutput=True)

    with TileContext(nc) as tc:
        with (
            tc.tile_pool(name="consts", bufs=1) as consts,
            tc.tile_pool(name="xtp", bufs=NJ) as xtp,
            tc.tile_pool(name="kqv", bufs=1) as kqv,
            tc.tile_pool(name="ptp", bufs=3) as ptp,
            tc.tile_pool(name="epi", bufs=4) as epi,
        ):
            # --- constants (w first: every matmul needs it) ---
            w_sb = consts.tile([128, NJ, 3 * HD], FP16)
            nc.sync.dma_start(out=w_sb[:], in_=w[:, :].rearrange("p (n m) -> p n m", n=NJ))
            bkq_sb = consts.tile([128, 1], F32)
            nc.sync.dma_start(out=bkq_sb[:], in_=bkq[:, :])
            bv_sb = consts.tile([64, 1], F32)
            nc.sync.dma_start(out=bv_sb[:], in_=bv[:, :])
            msk_sb = consts.tile([128, 128], FP16)
            nc.sync.dma_start(out=msk_sb[:], in_=msk[:, :])
            sel_sb = consts.tile([128, 64], FP16)
            nc.sync.dma_start(out=sel_sb[:], in_=sel[:, :])
            idh_sb = consts.tile([128, 128], FP16)
            nc.sync.dma_start(out=idh_sb[:], in_=idh[:, :])
            wu_sb = consts.tile([1, 512], FP16)
            nc.vector.memset(wu_sb[:], 1.0)

            # --- load x^T in 128-partition chunks ---
            xts = []
            for j in range(NJ):
                xt_t = xtp.tile([128, T], FP16, tag="xt")
                nc.sync.dma_start(out=xt_t[:], in_=xt[ts(j, 128), :])
                xts.append(xt_t)

            # --- qkv projection: all four 512-col groups accumulate at once
            # (j-outer, paced by the xt chunk DMAs); PE warms up on dummy
            # matmuls while the first chunks stream in ---
            kqT = kqv.tile([128, T], FP16)
            vT = kqv.tile([128, T], FP16)  # rows 64:128 zero-padded for transpose
            qT = kqv.tile([64, T], FP16)
            v1 = kqv.tile([128, NT, 80], FP16)  # [s, hd | ones | pad] per t-tile
            nc.vector.memset(vT[64:128, :], 0.0)
            with tc.tile_pool(name="psp", bufs=8, space=bass.MemorySpace.PSUM) as psp:
                wu_ps = psp.tile([128, 512], F32, tag="p")
                for r in range(6):
                    nc.tensor.matmul(wu_ps[:], wu_sb[:, 0:128], wu_sb[:], start=True, stop=True)
                kq_accs = [psp.tile([128, 512], F32, tag="p", name=f"kq_acc{n}") for n in range(NG)]
                v_accs = [psp.tile([64, 512], F32, tag="p", name=f"v_acc{n}") for n in range(NG)]
                for j in range(NJ):
                    first, last = j == 0, j == NJ - 1
                    for n in range(NG):
                        nc.tensor.matmul(
                            kq_accs[n][:], w_sb[:, j, 0:128], xts[j][:, ts(n, 512)],
                            start=first, stop=last,
                        )
                    for n in range(NG):
                        nc.tensor.matmul(
                            v_accs[n][:], w_sb[:, j, 128:192], xts[j][:, ts(n, 512)],
                            start=first, stop=last,
                        )
                for n in range(NG):
                    nc.vector.tensor_scalar_add(kqT[:, ts(n, 512)], kq_accs[n][:], bkq_sb[:])
                    nc.vector.tensor_scalar_add(vT[0:64, ts(n, 512)], v_accs[n][:], bv_sb[:])
                # q rows of kqT must move to base partition 0: PE row-extract
                for n in range(NG):
                    qp = psp.tile([64, 512], F32, tag="p", name=f"qp{n}")
                    nc.tensor.matmul(qp[:], sel_sb[:], kqT[:, ts(n, 512)], start=True, stop=True)
                    nc.vector.tensor_copy(qT[:, ts(n, 512)], qp[:])
                # v1 tiles via PE transpose of the zero-padded vT
                for i in range(NT):
                    tpv = psp.tile([128, 128], FP16, tag="p", name=f"tpv{i}")
                    nc.tensor.transpose(tpv[:], vT[:, ts(i, 128)], idh_sb[:])
                    nc.vector.tensor_copy(v1[:, i, 0:HD], tpv[:, 0:HD])
                    nc.vector.memset(v1[:, i, HD:HD + 1], 1.0)

            # --- attention (transposed PV accumulation) with per-group epilogue
            # interleaved into the loop ---
            with (
                tc.tile_pool(name="pso", bufs=4, space=bass.MemorySpace.PSUM) as pso,
                tc.tile_pool(name="pst", bufs=2, space=bass.MemorySpace.PSUM) as pst,
            ):
                outT_acc = [pso.tile([65, 512], F32, tag="o", name=f"outT_acc{g}") for g in range(NG)]
                for j in range(NT):
                    t0 = 128 * j
                    ptj = ptp.tile([128, T], FP16, tag="pt")
                    # scores^T in up-to-1024-col PSUM chunks, one exp per chunk
                    for h in range(t0 // 1024, 2):
                        base = 1024 * h
                        a, b2 = max(t0, base), base + 1024
                        stp = pst.tile([128, 1024], F32, tag="st")
                        for m in range(a // 512, b2 // 512):
                            pa, pb = max(a, 512 * m), 512 * (m + 1)
                            nc.tensor.matmul(
                                stp[:, pa - base:pb - base], kqT[0:64, ts(j, 128)], qT[:, pa:pb],
                                start=True, stop=True,
                            )
                        nc.scalar.activation(ptj[:, a:b2], stp[:, a - base:], EXP, scale=0.125)
                    # zero the below-diagonal (s > t) entries of the diagonal block
                    nc.vector.tensor_mul(ptj[:, ts(j, 128)], ptj[:, ts(j, 128)], msk_sb[:])
                    for g in range(j // 4, NG):
                        a = max(t0, 512 * g)
                        nc.tensor.matmul(
                            outT_acc[g][:, a - 512 * g:512], v1[:, j, 0:65], ptj[:, a:512 * (g + 1)],
                            start=(j == 0), stop=(j == 4 * g + 3),
                        )
                    if j % 4 == 3:
                        # t-group g is complete: transpose to [t, 65], normalize, store
                        g = j // 4
                        eo = epi.tile([128, 512], FP16, tag="eo")
                        nc.vector.memset(eo[64:128, :], 0.0)
                        nc.vector.tensor_copy(eo[0:65, :], outT_acc[g][:])
                        for l in range(4):
                            i = 4 * g + l
                            tp = pso.tile([128, 128], FP16, tag="o", name=f"tp{i}")
                            nc.tensor.transpose(tp[:], eo[:, ts(l, 128)], idh_sb[:])
                            rcp = epi.tile([128, 1], F32, tag="rcp")
                            nc.vector.reciprocal(rcp[:], tp[:, HD:HD + 1])
                            ob = epi.tile([128, HD], F32, tag="ob")
                            nc.vector.tensor_scalar_mul(ob[:], tp[:, 0:HD], rcp[:])
                            nc.sync.dma_start(out=out[ts(i, 128), :], in_=ob[:])
    nc.compile()
    return nc


_NC_CACHE = None


def _get_nc() -> bass.Bass:
    global _NC_CACHE
    if _NC_CACHE is None:
        _NC_CACHE = build_nc()
    return _NC_CACHE


def make_in_maps(x: np.ndarray, W: np.ndarray, b: np.ndarray) -> list[dict]:
    # pack W into the on-chip chunk layout: [128, NJ*192], contiguous DMA
    w16 = np.ascontiguousarray(
        W.astype(np.float16).reshape(NJ, 128, 3 * HD).transpose(1, 0, 2).reshape(128, NJ * 3 * HD)
    )
    bkq_h = np.ascontiguousarray(b[0:128].astype(np.float32).reshape(128, 1))
    bv_h = np.ascontiguousarray(b[128:192].astype(np.float32).reshape(64, 1))
    msk = np.triu(np.ones((128, 128), dtype=np.float16))        # keep s <= t
    sel = np.zeros((128, 64), dtype=np.float16)                  # rows 64:128 -> 0:64
    sel[np.arange(64) + 64, np.arange(64)] = 1.0
    idh = np.eye(128, dtype=np.float16)
    in_maps = []
    for core in range(N_CORES):
        xtc = np.ascontiguousarray(x[core].astype(np.float16).T)
        in_maps.append({"xt": xtc, "w": w16, "bkq": bkq_h, "bv": bv_h, "msk": msk, "sel": sel, "idh": idh})
    return in_maps


def run(x, W, b, trace: bool = False):
    """Returns (output [B, T, HD] fp32, BassKernelResults)."""
    x, W, b = np.asarray(x), np.asarray(W), np.asarray(b)
    nc = _get_nc()
    res = run_bass_kernel_spmd(nc, make_in_maps(x, W, b), list(range(N_CORES)), trace=trace)
    out = np.stack([res.results[i]["out"] for i in range(N_CORES)], axis=0)
    return out.astype(np.float32), res


def kernel(x, W, b) -> np.ndarray:
    out, _ = run(x, W, b)
    return out


# revision 16
# speedup vs baseline: 1.2837x; 1.0269x over previous
"""Single-head causal attention (B=8, T=2048, C=1024, head_dim=64) on 8 TRN2 NeuronCores.

Sharding: data-parallel over batch -- one batch element per core, qkv weights
replicated. Host prep per core: x[b] is transposed to [C, T] and cast to fp16
(PE streams fp16 at 1 cycle/row vs 4 for fp32; fp16's 11-bit mantissa keeps the
end-to-end error ~1e-3, and all PSUM accumulation stays fp32). W is pre-packed
on host into the SBUF chunk layout so its DMA is one contiguous transfer; the
tiny constant matrices (causal mask, row-extract selector, identity) also come
from host so no GPSIMD library load lands in the critical preamble.

Device schedule:
  kqT  = Wkq^T x^T + b_kq     [128, T]  (k rows 0:64, q rows 64:128; biases via
                                         an augmented ones-row K=1 chunk)
  qT   = rows 64:128 of kqT moved to base partition 0 via a PE row-extract
  vT   = Wv^T x^T + b_v       [128, T] (rows 64:128 zero) -> v1 [s, 65] tiles
                                        via PE transpose (+ones denom column)
  per s-chunk j:  ST_j = K_j Q^T  [128 s, t], t >= 128j only (causal),
                  P^T = exp(0.125*ST) in up-to-1024-col chunks, diagonal block
                  masked upper-tri, then  out_acc[i] += P^T_j[:, i]^T [v|1]_j
                  for every 128-row t-tile i >= j (output lands in NORMAL
                  [t, 65] orientation; 4 accumulators packed per PSUM bank)
  after j = 4m+3: t-tiles 4m..4m+3 are complete -> divide by the denominator
                  column and DMA out. No output transposes needed anywhere.
"""

import numpy as np

import concourse.bass as bass
import concourse.mybir as mybir
from concourse import bacc
from concourse.bass import ts
from concourse.bass_utils import run_bass_kernel_spmd
from concourse.tile import TileContext

B, T, C = 8, 2048, 1024
HD = 64
N_CORES = 8
NJ = C // 128  # contraction chunks for the qkv projection
NT = T // 128  # 128-row tiles along T
NG = T // 512  # 512-col groups along T
FP16 = mybir.dt.float16
CST_W = 8 * 192 + 2 + 128 + 64 + 128  # 1858
F32 = mybir.dt.float32
EXP = mybir.ActivationFunctionType.Exp


def build_nc() -> bass.Bass:
    nc = bacc.Bacc(None, target_bir_lowering=False)
    # w is pre-packed on host: [128, NJ*192] with w[p, j*192+m] = W[j*128+p, m]
    xt = nc.declare_dram_parameter("xt", [C, T], FP16, isOutput=False)
    # cst packs, per partition: NJ*192 w-chunk cols | bkq | bv | msk | sel | idh
    cst = nc.declare_dram_parameter("cst", [128, CST_W], FP16, isOutput=False)
    out = nc.declare_dram_parameter("out", [T, HD], F32, isOutput=True)

    with TileContext(nc) as tc:
        with (
            tc.tile_pool(name="consts", bufs=1) as consts,
            tc.tile_pool(name="xtp", bufs=NJ) as xtp,
            tc.tile_pool(name="kqv", bufs=1) as kqv,
            tc.tile_pool(name="ptp", bufs=3) as ptp,
            tc.tile_pool(name="epi", bufs=4) as epi,
        ):
            # --- constants: one contiguous DMA ---
            cst_sb = consts.tile([128, CST_W], FP16)
            nc.sync.dma_start(out=cst_sb[:], in_=cst[:, :])
            w_sb = cst_sb  # cols j*192 + [0:128) = Wkq_j, + [128:192) = Wv_j
            bkq_sb = cst_sb[:, 1536:1537]
            bv_sb = cst_sb[0:64, 1537:1538]
            msk_sb = cst_sb[:, 1538:1666]
            sel_sb = cst_sb[:, 1666:1730]
            idh_sb = cst_sb[:, 1730:1858]
            wu_sb = consts.tile([1, 512], FP16)
            nc.vector.memset(wu_sb[:], 1.0)
            bias32 = consts.tile([128, 2], F32)
            nc.vector.tensor_copy(bias32[:, 0:1], cst_sb[:, 1536:1537])
            nc.vector.tensor_copy(bias32[0:64, 1:2], cst_sb[0:64, 1537:1538])

            # --- load x^T in 128-partition chunks ---
            xts = []
            for j in range(NJ):
                xt_t = xtp.tile([128, T], FP16, tag="xt")
                eng = nc.sync if j % 2 == 0 else nc.scalar
                eng.dma_start(out=xt_t[:], in_=xt[ts(j, 128), :])
                xts.append(xt_t)

            # --- qkv projection: all four 512-col groups accumulate at once
            # (j-outer, paced by the xt chunk DMAs); PE warms up on dummy
            # matmuls while the first chunks stream in ---
            kqT = kqv.tile([128, T], FP16)
            vT = kqv.tile([128, T], FP16)  # rows 64:128 zero-padded for transpose
            qT = kqv.tile([64, T], FP16)
            v1 = kqv.tile([128, NT, 80], FP16)  # [s, hd | ones | pad] per t-tile
            nc.vector.memset(vT[64:128, :], 0.0)
            with tc.tile_pool(name="psp", bufs=8, space=bass.MemorySpace.PSUM) as psp:
                wu_ps = psp.tile([128, 512], F32, tag="p")
                for r in range(8):
                    nc.tensor.matmul(wu_ps[:], wu_sb[:, 0:128], wu_sb[:], start=True, stop=True)
                kq_accs = [psp.tile([128, 512], F32, tag="p", name=f"kq_acc{n}") for n in range(NG)]
                v_accs = [psp.tile([64, 512], F32, tag="p", name=f"v_acc{n}") for n in range(NG)]
                for j in range(NJ):
                    first, last = j == 0, j == NJ - 1
                    for n in range(NG):
                        nc.tensor.matmul(
                            kq_accs[n][:], w_sb[:, j * 192:j * 192 + 128], xts[j][:, ts(n, 512)],
                            start=first, stop=last,
                        )
                    for n in range(NG):
                        nc.tensor.matmul(
                            v_accs[n][:], w_sb[:, j * 192 + 128:j * 192 + 192], xts[j][:, ts(n, 512)],
                            start=first, stop=last,
                        )
                for n in range(NG):
                    nc.vector.tensor_scalar_add(kqT[:, ts(n, 512)], kq_accs[n][:], bias32[:, 0:1])
                    nc.vector.tensor_scalar_add(vT[0:64, ts(n, 512)], v_accs[n][:], bias32[0:64, 1:2])
                # q rows of kqT must move to base partition 0: PE row-extract
                for n in range(NG):
                    qp = psp.tile([64, 512], F32, tag="p", name=f"qp{n}")
                    nc.tensor.matmul(qp[:], sel_sb, kqT[:, ts(n, 512)], start=True, stop=True)
                    nc.vector.tensor_copy(qT[:, ts(n, 512)], qp[:])
                # v1 tiles via PE transpose of the zero-padded vT
                for i in range(NT):
                    tpv = psp.tile([128, 128], FP16, tag="p", name=f"tpv{i}")
                    nc.tensor.transpose(tpv[:], vT[:, ts(i, 128)], idh_sb)
                    nc.vector.tensor_copy(v1[:, i, 0:HD], tpv[:, 0:HD])
                    nc.vector.memset(v1[:, i, HD:HD + 1], 1.0)

            # --- attention (transposed PV accumulation) with per-group epilogue
            # interleaved into the loop ---
            with (
                tc.tile_pool(name="pso", bufs=4, space=bass.MemorySpace.PSUM) as pso,
                tc.tile_pool(name="pst", bufs=2, space=bass.MemorySpace.PSUM) as pst,
            ):
                outT_acc = [pso.tile([65, 512], F32, tag="o", name=f"outT_acc{g}") for g in range(NG)]
                for j in range(NT):
                    t0 = 128 * j
                    ptj = ptp.tile([128, T], FP16, tag="pt")
                    # scores^T in up-to-1024-col PSUM chunks, one exp per chunk
                    for h in range(t0 // 1024, 2):
                        base = 1024 * h
                        a, b2 = max(t0, base), base + 1024
                        stp = pst.tile([128, 1024], F32, tag="st")
                        for m in range(a // 512, b2 // 512):
                            pa, pb = max(a, 512 * m), 512 * (m + 1)
                            nc.tensor.matmul(
                                stp[:, pa - base:pb - base], kqT[0:64, ts(j, 128)], qT[:, pa:pb],
                                start=True, stop=True,
                            )
                        nc.scalar.activation(ptj[:, a:b2], stp[:, a - base:], EXP, scale=0.125)
                    # zero the below-diagonal (s > t) entries of the diagonal block
                    nc.vector.tensor_mul(ptj[:, ts(j, 128)], ptj[:, ts(j, 128)], msk_sb)
                    for g in range(j // 4, NG):
                        a = max(t0, 512 * g)
                        nc.tensor.matmul(
                            outT_acc[g][:, a - 512 * g:512], v1[:, j, 0:65], ptj[:, a:512 * (g + 1)],
                            start=(j == 0), stop=(j == 4 * g + 3),
                        )
                    if j % 4 == 3:
                        # t-group g is complete: transpose to [t, 65], normalize, store
                        g = j // 4
                        eo = epi.tile([128, 512], FP16, tag="eo")
                        nc.vector.memset(eo[64:128, :], 0.0)
                        nc.vector.tensor_copy(eo[0:65, :], outT_acc[g][:])
                        for l in range(4):
                            i = 4 * g + l
                            tp = pso.tile([128, 128], FP16, tag="o", name=f"tp{i}")
                            nc.tensor.transpose(tp[:], eo[:, ts(l, 128)], idh_sb)
                            rcp = epi.tile([128, 1], F32, tag="rcp")
                            nc.vector.reciprocal(rcp[:], tp[:, HD:HD + 1])
                            ob = epi.tile([128, HD], F32, tag="ob")
                            nc.vector.tensor_scalar_mul(ob[:], tp[:, 0:HD], rcp[:])
                            oeng = nc.sync if l % 2 == 0 else nc.scalar
                            oeng.dma_start(out=out[ts(i, 128), :], in_=ob[:])
    nc.compile()
    return nc


_NC_CACHE = None


def _get_nc() -> bass.Bass:
    global _NC_CACHE
    if _NC_CACHE is None:
        _NC_CACHE = build_nc()
    return _NC_CACHE


def make_in_maps(x: np.ndarray, W: np.ndarray, b: np.ndarray) -> list[dict]:
    cst = np.zeros((128, CST_W), dtype=np.float16)
    # w chunks: cst[p, j*192+m] = W[j*128+p, m]
    cst[:, :NJ * 3 * HD] = (
        W.astype(np.float16).reshape(NJ, 128, 3 * HD).transpose(1, 0, 2).reshape(128, NJ * 3 * HD)
    )
    cst[:, 1536] = b[0:128].astype(np.float16)
    cst[0:64, 1537] = b[128:192].astype(np.float16)
    cst[:, 1538:1666] = np.triu(np.ones((128, 128), dtype=np.float16))  # keep s <= t
    sel = np.zeros((128, 64), dtype=np.float16)
    sel[np.arange(64) + 64, np.arange(64)] = 1.0
    cst[:, 1666:1730] = sel
    cst[:, 1730:1858] = np.eye(128, dtype=np.float16)
    cst = np.ascontiguousarray(cst)
    in_maps = []
    for core in range(N_CORES):
        xtc = np.ascontiguousarray(x[core].astype(np.float16).T)
        in_maps.append({"xt": xtc, "cst": cst})
    return in_maps


def run(x, W, b, trace: bool = False):
    """Returns (output [B, T, HD] fp32, BassKernelResults)."""
    x, W, b = np.asarray(x), np.asarray(W), np.asarray(b)
    nc = _get_nc()
    res = run_bass_kernel_spmd(nc, make_in_maps(x, W, b), list(range(N_CORES)), trace=trace)
    out = np.stack([res.results[i]["out"] for i in range(N_CORES)], axis=0)
    return out.astype(np.float32), res


def kernel(x, W, b) -> np.ndarray:
    out, _ = run(x, W, b)
    return out


# revision 18
# speedup vs baseline: 1.5369x; 1.1972x over previous
"""Single-head causal attention (B=8, T=2048, C=1024, head_dim=64) on 8 TRN2 NeuronCores.

Sharding: data-parallel over batch -- one batch element per core, qkv weights
replicated. Host prep per core: x[b] is transposed to [C, T] and cast to fp16
(PE streams fp16 at 1 cycle/row vs 4 for fp32; fp16's 11-bit mantissa keeps the
end-to-end error ~1e-3, and all PSUM accumulation stays fp32). W is pre-packed
on host into the SBUF chunk layout so its DMA is one contiguous transfer; the
tiny constant matrices (causal mask, row-extract selector, identity) also come
from host so no GPSIMD library load lands in the critical preamble.

Device schedule:
  kqT  = Wkq^T x^T + b_kq     [128, T]  (k rows 0:64, q rows 64:128; biases via
                                         an augmented ones-row K=1 chunk)
  qT   = rows 64:128 of kqT moved to base partition 0 via a PE row-extract
  vT   = Wv^T x^T + b_v       [128, T] (rows 64:128 zero) -> v1 [s, 65] tiles
                                        via PE transpose (+ones denom column)
  per s-chunk j:  ST_j = K_j Q^T  [128 s, t], t >= 128j only (causal),
                  P^T = exp(0.125*ST) in up-to-1024-col chunks, diagonal block
                  masked upper-tri, then  out_acc[i] += P^T_j[:, i]^T [v|1]_j
                  for every 128-row t-tile i >= j (output lands in NORMAL
                  [t, 65] orientation; 4 accumulators packed per PSUM bank)
  after j = 4m+3: t-tiles 4m..4m+3 are complete -> divide by the denominator
                  column and DMA out. No output transposes needed anywhere.
"""

import numpy as np

import concourse.bass as bass
import concourse.mybir as mybir
from concourse import bacc
from concourse.bass import ts
from concourse.bass_utils import run_bass_kernel_spmd
from concourse.tile import TileContext

B, T, C = 8, 2048, 1024
HD = 64
N_CORES = 8
NJ = C // 128  # contraction chunks for the qkv projection
NT = T // 128  # 128-row tiles along T
NG = T // 512  # 512-col groups along T
FP16 = mybir.dt.float16
CST_W = 8 * 192 + 2 + 128 + 64 + 128  # 1858
F32 = mybir.dt.float32
EXP = mybir.ActivationFunctionType.Exp


def build_nc() -> bass.Bass:
    nc = bacc.Bacc(None, target_bir_lowering=False)
    # w is pre-packed on host: [128, NJ*192] with w[p, j*192+m] = W[j*128+p, m]
    xt = nc.declare_dram_parameter("xt", [C, T], FP16, isOutput=False)
    # cst packs, per partition: NJ*192 w-chunk cols | bkq | bv | msk | sel | idh
    cst = nc.declare_dram_parameter("cst", [128, CST_W], FP16, isOutput=False)
    out = nc.declare_dram_parameter("out", [T, HD], F32, isOutput=True)

    with TileContext(nc) as tc:
        with (
            tc.tile_pool(name="consts", bufs=1) as consts,
            tc.tile_pool(name="xtp", bufs=NJ) as xtp,
            tc.tile_pool(name="kqv", bufs=1) as kqv,
            tc.tile_pool(name="ptp", bufs=3) as ptp,
            tc.tile_pool(name="epi", bufs=4) as epi,
        ):
            # --- constants: one contiguous DMA ---
            cst_sb = consts.tile([128, CST_W], FP16)
            nc.scalar.dma_start(out=cst_sb[:], in_=cst[:, :])
            w_sb = cst_sb  # cols j*192 + [0:128) = Wkq_j, + [128:192) = Wv_j
            bkq_sb = cst_sb[:, 1536:1537]
            bv_sb = cst_sb[0:64, 1537:1538]
            msk_sb = cst_sb[:, 1538:1666]
            sel_sb = cst_sb[:, 1666:1730]
            idh_sb = cst_sb[:, 1730:1858]
            wu_sb = consts.tile([1, 512], FP16)
            nc.vector.memset(wu_sb[:], 1.0)
            bias32 = consts.tile([128, 2], F32)
            nc.vector.tensor_copy(bias32[:, 0:1], cst_sb[:, 1536:1537])
            nc.vector.tensor_copy(bias32[0:64, 1:2], cst_sb[0:64, 1537:1538])

            # --- load x^T in 128-partition chunks ---
            xts = []
            for j in range(NJ):
                xt_t = xtp.tile([128, T], FP16, tag="xt")
                eng = nc.sync if j % 2 == 0 else nc.scalar
                eng.dma_start(out=xt_t[:], in_=xt[ts(j, 128), :])
                xts.append(xt_t)

            # --- qkv projection: all four 512-col groups accumulate at once
            # (j-outer, paced by the xt chunk DMAs); PE warms up on dummy
            # matmuls while the first chunks stream in ---
            kqT = kqv.tile([128, T], FP16)
            vT = kqv.tile([128, T], FP16)  # rows 64:128 zero-padded for transpose
            qT = kqv.tile([64, T], FP16)
            v1 = kqv.tile([128, NT, 80], FP16)  # [s, hd | ones | pad] per t-tile
            nc.vector.memset(vT[64:128, :], 0.0)
            with tc.tile_pool(name="psp", bufs=8, space=bass.MemorySpace.PSUM) as psp:
                wu_ps = psp.tile([128, 512], F32, tag="p")
                for r in range(11):
                    nc.tensor.matmul(wu_ps[:], wu_sb[:, 0:128], wu_sb[:], start=True, stop=True)
                kq_accs = [psp.tile([128, 512], F32, tag="p", name=f"kq_acc{n}") for n in range(NG)]
                v_accs = [psp.tile([64, 512], F32, tag="p", name=f"v_acc{n}") for n in range(NG)]
                for j in range(NJ):
                    first, last = j == 0, j == NJ - 1
                    for n in range(NG):
                        nc.tensor.matmul(
                            kq_accs[n][:], w_sb[:, j * 192:j * 192 + 128], xts[j][:, ts(n, 512)],
                            start=first, stop=last,
                        )
                    for n in range(NG):
                        nc.tensor.matmul(
                            v_accs[n][:], w_sb[:, j * 192 + 128:j * 192 + 192], xts[j][:, ts(n, 512)],
                            start=first, stop=last,
                        )
                # keep PE busy across the DVE copy chain so HAM stays warm
                for r in range(6):
                    nc.tensor.matmul(wu_ps[:], wu_sb[:, 0:128], wu_sb[:], start=True, stop=True)
                for n in range(NG):
                    nc.vector.tensor_scalar_add(kqT[:, ts(n, 512)], kq_accs[n][:], bias32[:, 0:1])
                    nc.vector.tensor_scalar_add(vT[0:64, ts(n, 512)], v_accs[n][:], bias32[0:64, 1:2])
                # q rows of kqT must move to base partition 0: PE row-extract
                for n in range(NG):
                    qp = psp.tile([64, 512], F32, tag="p", name=f"qp{n}")
                    nc.tensor.matmul(qp[:], sel_sb, kqT[:, ts(n, 512)], start=True, stop=True)
                    nc.vector.tensor_copy(qT[:, ts(n, 512)], qp[:])
                # v1 tiles via PE transpose of the zero-padded vT
                for i in range(NT):
                    tpv = psp.tile([128, 128], FP16, tag="p", name=f"tpv{i}")
                    nc.tensor.transpose(tpv[:], vT[:, ts(i, 128)], idh_sb)
                    nc.vector.tensor_copy(v1[:, i, 0:HD], tpv[:, 0:HD])
                    nc.vector.memset(v1[:, i, HD:HD + 1], 1.0)

            # --- attention, t-group outer: one outT accumulator live at a time,
            # ST pieces for two s-chunks share a [128,1024] PSUM tile and one exp ---
            with (
                tc.tile_pool(name="pso", bufs=2, space=bass.MemorySpace.PSUM) as pso,
                tc.tile_pool(name="pst", bufs=3, space=bass.MemorySpace.PSUM) as pst,
            ):
                for g in range(NG):
                    gb = 512 * g
                    jmax = 4 * g + 3
                    acc = pso.tile([65, 512], F32, tag="o", name=f"outT_acc{g}")
                    for p in range(2 * g + 2):
                        stp = pst.tile([128, 1024], F32, tag="st")
                        ptt = ptp.tile([128, 1024], FP16, tag="pt")
                        for jj in (2 * p, 2 * p + 1):
                            col = 512 * (jj - 2 * p)
                            a = max(128 * jj, gb)
                            nc.tensor.matmul(
                                stp[:, col + a - gb:col + 512],
                                kqT[0:64, ts(jj, 128)], qT[:, a:gb + 512],
                                start=True, stop=True,
                            )
                        if 2 * p >= 4 * g:
                            # pair contains diagonal pieces: exp only written spans
                            for jj in (2 * p, 2 * p + 1):
                                col = 512 * (jj - 2 * p)
                                a = max(128 * jj, gb)
                                nc.scalar.activation(
                                    ptt[:, col + a - gb:col + 512],
                                    stp[:, col + a - gb:col + 512], EXP, scale=0.125,
                                )
                        else:
                            nc.scalar.activation(ptt[:], stp[:], EXP, scale=0.125)
                        for jj in (2 * p, 2 * p + 1):
                            col = 512 * (jj - 2 * p)
                            a = max(128 * jj, gb)
                            if jj >= 4 * g:
                                # diagonal block: zero the below-diagonal entries
                                nc.vector.tensor_mul(
                                    ptt[:, col + a - gb:col + a - gb + 128],
                                    ptt[:, col + a - gb:col + a - gb + 128], msk_sb,
                                )
                            nc.tensor.matmul(
                                acc[:, a - gb:512], v1[:, jj, 0:65],
                                ptt[:, col + a - gb:col + 512],
                                start=(jj == 0), stop=(jj == jmax),
                            )
                    # epilogue: transpose to [t, 65], normalize, store
                    eo = epi.tile([128, 512], FP16, tag="eo")
                    nc.vector.memset(eo[64:128, :], 0.0)
                    nc.vector.tensor_copy(eo[0:65, :], acc[:])
                    for l in range(4):
                        i = 4 * g + l
                        tp = pso.tile([128, 128], FP16, tag="o", name=f"tp{i}")
                        nc.tensor.transpose(tp[:], eo[:, ts(l, 128)], idh_sb)
                        rcp = epi.tile([128, 1], F32, tag="rcp")
                        nc.vector.reciprocal(rcp[:], tp[:, HD:HD + 1])
                        ob = epi.tile([128, HD], F32, tag="ob")
                        nc.vector.tensor_scalar_mul(ob[:], tp[:, 0:HD], rcp[:])
                        oeng = nc.sync if l % 2 == 0 else nc.scalar
                        oeng.dma_start(out=out[ts(i, 128), :], in_=ob[:])
    nc.compile()
    return nc


_NC_CACHE = None


def _get_nc() -> bass.Bass:
    global _NC_CACHE
    if _NC_CACHE is None:
        _NC_CACHE = build_nc()
    return _NC_CACHE


def make_in_maps(x: np.ndarray, W: np.ndarray, b: np.ndarray) -> list[dict]:
    cst = np.zeros((128, CST_W), dtype=np.float16)
    # w chunks: cst[p, j*192+m] = W[j*128+p, m]
    cst[:, :NJ * 3 * HD] = (
        W.astype(np.float16).reshape(NJ, 128, 3 * HD).transpose(1, 0, 2).reshape(128, NJ * 3 * HD)
    )
    cst[:, 1536] = b[0:128].astype(np.float16)
    cst[0:64, 1537] = b[128:192].astype(np.float16)
    cst[:, 1538:1666] = np.triu(np.ones((128, 128), dtype=np.float16))  # keep s <= t
    sel = np.zeros((128, 64), dtype=np.float16)
    sel[np.arange(64) + 64, np.arange(64)] = 1.0
    cst[:, 1666:1730] = sel
    cst[:, 1730:1858] = np.eye(128, dtype=np.float16)
    cst = np.ascontiguousarray(cst)
    in_maps = []
    for core in range(N_CORES):
        xtc = np.ascontiguousarray(x[core].astype(np.float16).T)
        in_maps.append({"xt": xtc, "cst": cst})
    return in_maps


def run(x, W, b, trace: bool = False):
    """Returns (output [B, T, HD] fp32, BassKernelResults)."""
    x, W, b = np.asarray(x), np.asarray(W), np.asarray(b)
    nc = _get_nc()
    res = run_bass_kernel_spmd(nc, make_in_maps(x, W, b), list(range(N_CORES)), trace=trace)
    out = np.stack([res.results[i]["out"] for i in range(N_CORES)], axis=0)
    return out.astype(np.float32), res


def kernel(x, W, b) -> np.ndarray:
    out, _ = run(x, W, b)
    return out
